# revision 43
# baseline (speedup 1.0000x reference)
"""Trainium2 Bass kernel for the Capsule routing layer (nn_Capsule_49658411876931).

Math (see reference):
    u_hat[b,j,i,d] = sum_k W[j,i,d,k] * x[b,i,k]
    b0 = 0
    for r in 0..2:
        c = softmax(b, axis=j)
        s[b,j,d] = sum_i c[b,j,i] u_hat[b,j,i,d]
        v = squash(s)  (over d)
        if r < 2: b += sum_d u_hat[b,j,i,d] v[b,j,d]
    return v  [B, J, D]

Sharding: input-capsule axis I=2048 split over 8 cores (I_LOC=256).  The only
cross-core communication is an AllReduce of the partial s [B, J*D] = 64 KB per
routing iteration.

Per-core layouts (P = SBUF partition index), i_local = g*16 + r*4 + c:
  u_hat "C"  : [P = 32*c + b, free = (g, r, d, j)]  fp16
  logits bl  : [P = 32*c + b, free = (g, r, j)]     fp32, lives in PSUM
All big reductions run on the PE array (cheap), not the DVE:
  - iter-0 s: direct matmul over (i,k) chunks (c is uniform 1/J).
  - s-step: lhsT = strip-collapse selector [128, 32], moving = pi slices
    [128, (d,j)] per (g,r), PSUM-accumulated over the 64 slices.
  - agreement: lhsT = identity [128, 128], moving = pi2 slices [128, (g,r,j)]
    per d, PSUM-accumulated over d directly into the logits PSUM region
    (and across routing iterations: b += ...).
  - phase-1 u_hat: contraction over (c,k)=32 rows at 4 row-strip tile
    positions, stationary = c'-selector-expanded x [32, 128], moving =
    W [32, (d,j)]; W passes through the PE exactly once, as fp16.
The agreement -> softmax -> s-step chain is pipelined at g-block granularity
(4 groups per block): PE accumulates block q's agreement while the DVE works
on block q's softmax products, keeping DVE/PE/ACT all busy.  The remaining
DVE work is the per-sample elementwise mults (pi = C*c, pi2 = C*v) at 2x fp16
throughput — the structural floor (both factors are per-sample, so the PE
cannot fuse them).  The s-step runs as 4 concurrent per-r accumulation
chains on separate 32-column strips of the PE array, collapsed by one final
selector matmul.  AllReduce payloads are fp16.  The first AllReduce and the
u_hat build are both hidden under the cross-core NEFF start-skew barrier.
"""

import numpy as np
import ml_dtypes

import concourse.bass as bass
import concourse.tile as tile
from concourse import bacc, mybir
from concourse.bass_utils import run_bass_kernel_spmd

F32 = mybir.dt.float32
F16 = mybir.dt.float16  # fp16: 11-bit mantissa, ample range here
U32 = mybir.dt.uint32
Alu = mybir.AluOpType
Act = mybir.ActivationFunctionType

B, I, K = 32, 2048, 8
J, D = 32, 16
JD = J * D                     # 512
NCORES = 8
I_LOC = I // NCORES            # 256
NG = I_LOC // 16               # 16 groups of 16 input capsules per core
NCH = I_LOC * K // 128         # 16 contraction chunks of 128 for (i,k)
ROUTINGS = 3
EPS = 1e-7

_CACHE = {}
import os
DEBUG_STAGE = os.environ.get("KSTAGE", "")


def _build():
    nc = bacc.Bacc("TRN2", target_bir_lowering=False, debug=False, num_devices=NCORES)

    # NOTE: the (i,k)-chunk layout needed by the iter-0 matmul and the
    # (r,c,k)-strip layout needed by phase-1 are the SAME byte layout
    # ((r,c,k) flattens to r*32+c*8+k = (i%16)*8+k), so one W tensor serves
    # both.
    wm_in = nc.dram_tensor("wm", [128, NG, JD], F16, kind="ExternalInput")
    xsel_in = nc.dram_tensor("xsel", [128, NG, 128], F16, kind="ExternalInput")
    xs0_in = nc.dram_tensor("xs0", [128, NCH, B], F16, kind="ExternalInput")
    v_out = nc.dram_tensor("v", [B, J, D], F32, kind="ExternalOutput")

    # f32 constants: selT (s broadcast to all partitions), rsqrt magic numbers
    cf32_np = np.zeros((128, 192), np.float32)
    selT_np = np.zeros((B, 128), np.float32)
    selT_np[np.arange(128) % B, np.arange(128)] = 1.0
    cf32_np[0:B, 0:128] = selT_np
    cf32_np[:, 128:160] = np.full((128, J), 0x5F3759DF, np.uint32).view(np.float32)
    cf32_np[:, 160:192] = np.full((128, J), 1, np.uint32).view(np.float32)
    cf32_dram = nc.inline_tensor(cf32_np, "cf32")

    # f16 constants: sel (strip collapse) + identity (d-accumulate
    # pass-through) + selT (broadcast to all partitions)
    cf16_np = np.zeros((128, 288), np.float16)
    cf16_np[np.arange(128), np.arange(128) % B] = 1.0           # sel [128, 32]
    cf16_np[np.arange(128), 32 + np.arange(128)] = 1.0          # ident [128,128]
    cf16_np[np.arange(128) % B, 160 + np.arange(128)] = 1.0     # selT [32,128]
    cf16_dram = nc.inline_tensor(cf16_np, "cf16")



    with tile.TileContext(nc) as tc:
        with (
            tc.tile_pool(name="persist", bufs=1) as pp,
            tc.tile_pool(name="small", bufs=1) as sp,
            tc.tile_pool(name="dram", bufs=1, space="DRAM") as dp,
            tc.tile_pool(name="psA", bufs=1, space="PSUM") as psA,
        ):
            # ---- persistent SBUF tensors ----
            C = pp.tile([128, NG, 4, D, J], F16)        # u_hat
            p_t = pp.tile([128, NG, 4, J], F32)         # exp(bl)
            c_sb = pp.tile([128, NG, 4, J], F16)        # softmax coefficients
            v_rep = pp.tile([128, D, J], F16)           # v replicated to all partitions
            s_rep = pp.tile([128, D, J], F32)           # s replicated (squash input)
            wm = pp.tile([128, NG, JD], F16)
            xs0 = pp.tile([128, NCH, B], F16)
            xsel = pp.tile([128, NG, 128], F16)
            cf32 = pp.tile([128, 192], F32)
            cf16 = pp.tile([128, 288], F16)
            S = pp.tile([128, NG, 4], F32)
            Sr = pp.tile([128, NG, 4], F32)

            magic = cf32[:, 128:160].bitcast(U32)
            oneu = cf32[:, 160:192].bitcast(U32)
            sel = cf16[:, 0:32]
            ident = cf16[:, 32:160]
            selT = cf16[0:B, 160:288]

            # small DMAs first so the iter-0 matmuls can start early; wm in
            # four chunks so the first matmuls don't wait for the whole 2MB
            nc.sync.dma_start(cf32[:], cf32_dram[:])
            nc.sync.dma_start(cf16[:], cf16_dram[:])
            nc.sync.dma_start(xs0[:], xs0_in[:])
            tc.strict_bb_all_engine_barrier()
            for q in range(4):
                nc.sync.dma_start(
                    wm[:, 4 * q : 4 * q + 4], wm_in[:, 4 * q : 4 * q + 4]
                )
            nc.sync.dma_start(xsel[:], xsel_in[:])

            # PSUM allocations (eager, so the later phase-1 pool stacks on top)
            s_ps = psA.tile([B, JD], F32, tag="s_ps")
            srep_ps = psA.tile([128, JD], F32, tag="srep_ps")
            s4_ps = psA.tile([128, JD], F32, tag="s4_ps")

            # ---- iter-0 s: direct matmul, c uniform (1/J folded into xs0) ----
            for ch in range(NCH):
                nc.tensor.matmul(
                    s_ps[:], xs0[:, ch, :], wm[:, ch, :],
                    start=(ch == 0), stop=(ch == NCH - 1),
                )

            def all_reduce_s(it):
                cc_in = dp.tile([B, JD], F16, tag="cc_in")
                cc_out = dp.tile([B, JD], F16, tag="cc_out", addr_space="Shared")
                s_glob = sp.tile([B, JD], F16, tag="s_glob")
                s_loc = sp.tile([B, JD], F16, tag="s_loc")
                nc.scalar.copy(s_loc[:], s_ps[:])
                nc.gpsimd.dma_start(cc_in[:], s_loc[:])
                nc.gpsimd.collective_compute(
                    "AllReduce",
                    Alu.add,
                    replica_groups=[list(range(NCORES))],
                    ins=[cc_in.opt()],
                    outs=[cc_out.opt()],
                )
                nc.gpsimd.dma_start(s_glob[:], cc_out[:])
                return s_glob

            s_glob = all_reduce_s(0)

            # ---- phase 1 (overlaps the AllReduce): u_hat via (c,k)-contraction
            with (
                tc.tile_pool(name="ph1", bufs=2, space="PSUM") as ph1,
            ):
                for g in range(NG):
                    for rp in range(2):
                        ps1 = ph1.tile([128, 2, JD], F32, tag="ps1")
                        for rr in range(2):
                            r = 2 * rp + rr
                            nc.tensor.matmul(
                                ps1[:, rr, :],
                                xsel[32 * r : 32 * r + 32, g, :],
                                wm[32 * r : 32 * r + 32, g, :],
                                tile_position=(32 * r, 0),
                            )
                        # evacuate [128, 2, (d,j)] -> C[:, g, 2rp:2rp+2, :, :]
                        dst = C[:, g, 2 * rp : 2 * rp + 2].rearrange(
                            "p r d j -> p r (d j)"
                        )
                        if rp == 0:
                            nc.scalar.copy(dst, ps1[:])
                        else:
                            nc.vector.tensor_copy(dst, ps1[:])

            # ---- squash on replicated s: v_rep [128, D, J] f16 ----
            # (all partitions redundantly; row p carries b = p%32)
            def squash(s_glob, last):
                nc.tensor.matmul(srep_ps[:], selT, s_glob[:])
                nc.scalar.copy(s_rep.rearrange("p d j -> p (d j)"), srep_ps[:])
                sq = sp.tile([128, D, J], F32, tag="sq")
                nc.vector.tensor_tensor(sq[:], s_rep[:], s_rep[:], op=Alu.mult)
                n2 = sp.tile([128, J], F32, tag="n2")
                nc.vector.tensor_reduce(
                    n2[:],
                    sq.rearrange("p d j -> p j d"),
                    axis=mybir.AxisListType.X,
                    op=Alu.add,
                )
                n2e = sp.tile([128, J], F32, tag="n2e")
                nc.vector.tensor_scalar_add(n2e[:], n2[:], EPS)
                # fast inverse sqrt + 2 Newton steps (DVE only, no ACT tables)
                xh = sp.tile([128, J], F32, tag="xh")
                nc.vector.tensor_scalar_mul(xh[:], n2e[:], 0.5)
                rsq = sp.tile([128, J], F32, tag="rsq")
                tmp = sp.tile([128, J], F32, tag="tmp")
                nc.vector.tensor_tensor(
                    tmp.bitcast(U32), n2e.bitcast(U32), oneu,
                    op=Alu.logical_shift_right,
                )
                nc.vector.tensor_tensor(
                    rsq.bitcast(U32), magic, tmp.bitcast(U32), op=Alu.subtract
                )
                for _ in range(2):
                    nc.vector.tensor_tensor(tmp[:], rsq[:], rsq[:], op=Alu.mult)
                    nc.vector.tensor_tensor(tmp[:], xh[:], tmp[:], op=Alu.mult)
                    nc.vector.tensor_scalar(
                        tmp[:], tmp[:], -1.0, 1.5, op0=Alu.mult, op1=Alu.add
                    )
                    nc.vector.tensor_tensor(rsq[:], rsq[:], tmp[:], op=Alu.mult)
                # factor = n2 / (1 + n2) * rsq
                fac = sp.tile([128, J], F32, tag="fac")
                nc.vector.tensor_scalar_add(tmp[:], n2[:], 1.0)
                nc.vector.reciprocal(fac[:], tmp[:])
                nc.vector.tensor_tensor(fac[:], fac[:], n2[:], op=Alu.mult)
                nc.vector.tensor_tensor(fac[:], fac[:], rsq[:], op=Alu.mult)
                if last:
                    # write v directly in [B, J, D] output order (strided)
                    v_jd = sp.tile([B, J, D], F32, tag="v_jd")
                    nc.vector.tensor_tensor(
                        v_jd.rearrange("b j d -> b d j"),
                        s_rep[0:B],
                        fac[0:B, None, :].broadcast_to([B, D, J]),
                        op=Alu.mult,
                    )
                    return v_jd
                nc.vector.tensor_tensor(
                    v_rep[:],
                    s_rep[:],
                    fac[:, None, :].broadcast_to([128, D, J]),
                    op=Alu.mult,
                )
                return None

            with (
                tc.tile_pool(name="prod", bufs=3) as prod,
                tc.tile_pool(name="psC", bufs=1, space="PSUM") as psC,
            ):
                # persistent routing logits bl [128, (g, r, j)] = 4 PSUM banks
                bl_ps = psC.tile([128, NG, 4, J], F32)
                for it in range(ROUTINGS - 1):
                    squash(s_glob, last=False)
                    # two-pass software pipeline: first all agreement blocks
                    # (DVE computes pi2(k+1) while the PE accumulates block
                    # k's d-sums), then all softmax + s-step blocks — the DVE
                    # never stalls on the agreement->exp feedback.
                    for blk in range(4):
                        g0 = 4 * blk
                        # d-major pi2 so the agreement matmul movings are
                        # contiguous [128, (g,r,j)] slices; computed in two
                        # d-halves so the PE starts accumulating during the
                        # second half's DVE work
                        pi2 = prod.tile([128, D, 16, J], F16, tag="pi2")
                        for h in range(2):
                            dh = 8 * h
                            nc.vector.tensor_tensor(
                                pi2[:, dh : dh + 8],
                                C[:, g0 : g0 + 4].rearrange(
                                    "p g r d j -> p d (g r) j"
                                )[:, dh : dh + 8],
                                v_rep[:, dh : dh + 8, None, :].broadcast_to(
                                    [128, 8, 16, J]
                                ),
                                op=Alu.mult,
                            )
                        for d in range(D):
                            nc.tensor.matmul(
                                bl_ps[:, g0 : g0 + 4, :, :],
                                ident,
                                pi2[:, d],
                                start=(it == 0 and d == 0),
                                stop=(d == D - 1),
                                skip_group_check=True,
                            )
                    for blk in range(4):
                        g0 = 4 * blk
                        # per-block softmax over j
                        nc.scalar.activation(
                            p_t[:, g0 : g0 + 4], bl_ps[:, g0 : g0 + 4], Act.Exp
                        )
                        nc.vector.tensor_reduce(
                            S[:, g0 : g0 + 4],
                            p_t[:, g0 : g0 + 4],
                            axis=mybir.AxisListType.X,
                            op=Alu.add,
                        )
                        nc.vector.reciprocal(Sr[:, g0 : g0 + 4], S[:, g0 : g0 + 4])
                        nc.vector.tensor_tensor(
                            c_sb[:, g0 : g0 + 4],
                            p_t[:, g0 : g0 + 4],
                            Sr[:, g0 : g0 + 4, :, None].broadcast_to(
                                [128, 4, 4, J]
                            ),
                            op=Alu.mult,
                        )
                        # pi = C * c (bcast over d); two TTs per block — the
                        # (g,r) dims merge so every AP fits the 3D formats,
                        # and the s-matmuls start during the second half
                        pi = prod.tile([128, 16, D, J], F16, tag="pi")
                        for h in range(2):
                            gr = 8 * h
                            nc.vector.tensor_tensor(
                                pi[:, gr : gr + 8],
                                C[:, g0 : g0 + 4]
                                .rearrange("p g r d j -> p (g r) d j")[
                                    :, gr : gr + 8
                                ],
                                c_sb[:, g0 : g0 + 4, :, None, :]
                                .rearrange("p g r d j -> p (g r) d j")[
                                    :, gr : gr + 8
                                ]
                                .broadcast_to([128, 8, D, J]),
                                op=Alu.mult,
                            )
                        # 4 concurrent accumulation chains, one per r on
                        # its own 32-column strip of the PE array
                        for g in range(g0, g0 + 4):
                            for r in range(4):
                                nc.tensor.matmul(
                                    s4_ps[32 * r : 32 * r + 32, :],
                                    sel,
                                    pi[:, 4 * (g - g0) + r].rearrange(
                                        "p d j -> p (d j)"
                                    ),
                                    start=(g == 0),
                                    stop=(g == NG - 1),
                                    tile_position=(0, 32 * r),
                                    skip_group_check=True,
                                )
                    # collapse the 4 r-strips: s_ps = sel.T @ s4
                    s4_sb = sp.tile([128, JD], F16, tag="s4_sb")
                    nc.scalar.copy(s4_sb[:], s4_ps[:])
                    nc.tensor.matmul(s_ps[:], sel, s4_sb[:])
                    s_glob = all_reduce_s(it + 1)

                v_jd = squash(s_glob, last=True)
                nc.sync.dma_start(v_out[:], v_jd[:])

    nc.compile()
    return nc


def _prep_inputs(x, W):
    """Per-core host-side sharding + layout prep (fp16)."""
    x16 = x.astype(np.float16)
    W16 = W.astype(np.float16)
    in_maps = []
    for m in range(NCORES):
        lo, hi = m * I_LOC, (m + 1) * I_LOC
        Wc = W16[:, lo:hi]                     # [J, 256, D, K]
        # wm[(r,c,k), g, (d,j)] = Wc[j, g*16+r*4+c, d, k]
        Wm = Wc.reshape(J, NG, 4, 4, D, K)     # j, g, r, c, d, k
        wm = np.ascontiguousarray(Wm.transpose(2, 3, 5, 1, 4, 0)).reshape(
            128, NG, JD
        )
        xc = x16[:, lo:hi, :]                  # [B, 256, K]
        # xsel[(r,c,k), g, (c',b)] = x[b, g*16+r*4+c, k] * [c == c']
        xg = xc.reshape(B, NG, 4, 4, K)        # b, g, r, c, k
        xsel = np.zeros((4, 4, K, NG, 4, B), np.float16)
        for c in range(4):
            xsel[:, c, :, :, c, :] = xg.transpose(2, 3, 4, 1, 0)[:, c]
        xsel = xsel.reshape(128, NG, 128)
        # xs0[(i16,k), ch, b] = x[b, ch*16+i16, k] / J
        xs = xc.reshape(B, NCH, 16, K).transpose(2, 3, 1, 0)  # i16, k, ch, b
        xs0 = np.ascontiguousarray(xs).reshape(128, NCH, B) * np.float16(1.0 / J)
        in_maps.append({"wm": wm, "xsel": xsel, "xs0": xs0})
    return in_maps


def run(inputs, trace=False):
    if "nc" not in _CACHE:
        _CACHE["nc"] = _build()
    nc = _CACHE["nc"]
    in_maps = _prep_inputs(inputs["x"], inputs["W"])
    bkr = run_bass_kernel_spmd(
        nc, in_maps, core_ids=list(range(NCORES)), trace=trace
    )
    out = bkr.results[0]["v"].astype(np.float32)
    return out, bkr


def kernel(x, W):
    out, _ = run({"x": np.asarray(x), "W": np.asarray(W)})
    return out
